# revision 14
# baseline (speedup 1.0000x reference)
"""Masked-softmax attention block on 8 TRN2 NeuronCores.

Math per batch element b (one NeuronCore each, B=8):
    q = x @ Wq ; k = x @ Wk ; v = x @ Wv          [S,D]
    s = (q @ k^T) / sqrt(D)                        [S,S]
    w = softmax(s * m) ; w = w * m ; w /= sum(w)   (m = mask in {0,1})
    out = w @ v                                    [S,D]

Identity used: final attention row i is
    p_j = m_j * exp(s_j)  (up to the shared max-shift), out = (p @ v)/sum(p)
Implemented as p_j = exp(u_j - BIAS) with u_j = (s_j/32 + BIAS)*m_j so that
masked lanes land on exp(-BIAS) ~ 9e-14 ~ 0 with no extra select pass; the
row-sum comes free from the ACT exp's accum_out and 1/den is folded into the
PV PSUM eviction as a per-partition scale.

All matmuls run in float32r (TF32; ~1.6e-4 matmul rel err, 4x fp32 rate).
Set MM_DTYPE = F32 below for full fp32 (4x slower on PE).
"""

import numpy as np

import concourse.bass as bass
import concourse.tile as tile
from concourse import bacc, mybir
from concourse.bass_utils import run_bass_kernel_spmd

F32 = mybir.dt.float32
F32R = mybir.dt.float32r
I32 = mybir.dt.int32

MM_DTYPE = F32R          # matmul operand dtype (F32R or F32)

B, S, E, D = 8, 2048, 1024, 1024
P = 128                  # partition dim
NS = S // P              # 16 s-tiles
NE = E // P              # 8 e-chunks
ND = D // P              # 8 d-chunks
KH = 2                   # k-halves in attention (1024 each)
BIAS = 30.0              # exp bias for the masked-softmax trick
SCALE = 1.0 / np.sqrt(D).astype(np.float32)


def build_program(repeat=1, ablate="none", mask_eng="gpsimd", qt_eng="sync", mask_dtype="int8", deep_bufs=False, evict_mix=True, big_dma=False):
    nc = bacc.Bacc("TRN2", target_bir_lowering=False)
    MT = MM_DTYPE

    x_d = nc.declare_dram_parameter("x", [S, E], F32, isOutput=False)
    MDT = I32 if mask_dtype == "int32" else mybir.dt.int8
    mask_d = nc.declare_dram_parameter("mask", [S, S], MDT, isOutput=False)
    wq_d = nc.declare_dram_parameter("Wq", [E, D], F32, isOutput=False)
    wk_d = nc.declare_dram_parameter("Wk", [E, D], F32, isOutput=False)
    wv_d = nc.declare_dram_parameter("Wv", [E, D], F32, isOutput=False)
    id_d = nc.declare_dram_parameter("ident", [P, P], F32, isOutput=False)
    out_d = nc.declare_dram_parameter("out", [S, D], F32, isOutput=True)

    qT_dram = nc.dram_tensor("qT_scratch", [D, S], MT)

    import contextlib
    with tile.TileContext(nc) as tc, contextlib.ExitStack() as _loop_ctx:
        if repeat > 1:
            _loop_ctx.enter_context(tc.For_i(0, repeat, 1))
        const_pool = tc.alloc_tile_pool(name="const", bufs=1)
        ident = const_pool.tile([P, P], F32, name="ident")
        nc.sync.dma_start(ident[:], id_d[:])
        nbias = const_pool.tile([P, 1], F32, name="nbias")
        nc.vector.memset(nbias[:], -BIAS)

        # persistent right-stack tensors (outlive xT on the left stack)
        v_pool = tc.alloc_tile_pool(name="vp", bufs=1, side="right")
        v_big = v_pool.tile([P, NS, D], MM_DTYPE, name="v_big")  # 64KB/part

        # ---------------- Phase T: xT[e,s] = x^T via PE transpose ----------
        xT_pool = tc.alloc_tile_pool(name="xT", bufs=1)
        xT = xT_pool.tile([P, NE, S], MT, name="xT")        # 64KB/part
        if big_dma:
            xload_pool = tc.alloc_tile_pool(name="xload", bufs=2)
            psT_pool = tc.alloc_tile_pool(name="psT", bufs=4, space="PSUM")
            x_view = x_d[:].rearrange("(b p) e -> p b e", p=P)
            for ib in range(NS // 4):
                xt4 = xload_pool.tile([P, 4, E], F32, name="xt4")
                nc.sync.dma_start(
                    xt4[:], x_view[:, ib * 4:(ib + 1) * 4, :]
                )
                for bi in range(4):
                    i = ib * 4 + bi
                    for g in range(2):
                        pt = psT_pool.tile([P, 4, P], F32, name="pt")
                        for jj in range(4):
                            j = g * 4 + jj
                            nc.tensor.transpose(
                                pt[:, jj, :], xt4[:, bi, j * P:(j + 1) * P],
                                ident[:]
                            )
                        nc.vector.tensor_copy(
                            xT[:, g * 4:(g + 1) * 4, i * P:(i + 1) * P], pt[:]
                        )
        else:
            xload_pool = tc.alloc_tile_pool(name="xload", bufs=3)
            psT_pool = tc.alloc_tile_pool(name="psT", bufs=4, space="PSUM")
            for i in range(NS):
                xt = xload_pool.tile([P, E], F32, name="xt")
                nc.sync.dma_start(xt[:], x_d[i * P:(i + 1) * P, :])
                for g in range(2):
                    pt = psT_pool.tile([P, 4, P], F32, name="pt")
                    for jj in range(4):
                        j = g * 4 + jj
                        nc.tensor.transpose(
                            pt[:, jj, :], xt[:, j * P:(j + 1) * P], ident[:]
                        )
                    nc.vector.tensor_copy(
                        xT[:, g * 4:(g + 1) * 4, i * P:(i + 1) * P], pt[:]
                    )
        psT_pool.release()
        xload_pool.release()

        # ---------------- Phase Q: qT[d,s] = Wq^T-proj, spilled to DRAM ----
        wq_pool = tc.alloc_tile_pool(name="wqp", bufs=1)
        wq = wq_pool.tile([P, NE, D], MT, name="wq")        # 32KB/part
        nc.sync.dma_start(
            wq[:], wq_d[:].rearrange("(j p) d -> p j d", p=P).bitcast(MT)
        )
        psQ_pool = tc.alloc_tile_pool(name="psQ", bufs=2, space="PSUM")
        qbuf_pool = tc.alloc_tile_pool(name="qbuf", bufs=2)
        for dt in range(ND):
            psq = psQ_pool.tile([P, S], F32, name="psq")    # 4 banks
            for e in range(NE):
                for sc in range(4):
                    nc.tensor.matmul(
                        psq[:, sc * 512:(sc + 1) * 512],
                        wq[:, e, dt * P:(dt + 1) * P],
                        xT[:, e, sc * 512:(sc + 1) * 512],
                        start=(e == 0), stop=(e == NE - 1),
                    )
            qb = qbuf_pool.tile([P, S], MT, name="qb")
            if evict_mix and dt % 2 == 1:
                nc.vector.tensor_copy(qb[:], psq[:])
            else:
                nc.scalar.copy(qb[:], psq[:])
            nc.sync.dma_start(qT_dram[dt * P:(dt + 1) * P, :], qb[:])
        qbuf_pool.release()
        psQ_pool.release()
        wq_pool.release()

        # ---------------- Phase V: v[s,d] ----------------------------------
        wv_pool = tc.alloc_tile_pool(name="wvp", bufs=1)
        wv = wv_pool.tile([P, NE, D], MT, name="wv")        # 32KB/part
        nc.sync.dma_start(
            wv[:], wv_d[:].rearrange("(j p) d -> p j d", p=P).bitcast(MT)
        )
        psV_pool = tc.alloc_tile_pool(name="psV", bufs=2, space="PSUM")
        for st in range(NS):
            psv = psV_pool.tile([P, D], F32, name="psv")    # 2 banks
            for e in range(NE):
                for dh in range(2):
                    nc.tensor.matmul(
                        psv[:, dh * 512:(dh + 1) * 512],
                        xT[:, e, st * P:(st + 1) * P],
                        wv[:, e, dh * 512:(dh + 1) * 512],
                        start=(e == 0), stop=(e == NE - 1),
                    )
            if evict_mix and st % 2 == 1:
                nc.vector.tensor_copy(v_big[:, st, :], psv[:])
            else:
                nc.scalar.copy(v_big[:, st, :], psv[:])
        psV_pool.release()
        wv_pool.release()

        # ---------------- Phase K: kT[d,s], Wk streamed per d-tile ---------
        kT_pool = tc.alloc_tile_pool(name="kTp", bufs=1, side="right")
        kT = kT_pool.tile([P, ND, S], MT, name="kT")        # 64KB/part
        wk_pool = tc.alloc_tile_pool(name="wkp", bufs=2)
        psK_pool = tc.alloc_tile_pool(name="psK", bufs=2, space="PSUM")
        for dt in range(ND):
            wk = wk_pool.tile([P, NE, P], MT, name="wk")    # 4KB/part
            nc.sync.dma_start(
                wk[:],
                wk_d[:, dt * P:(dt + 1) * P]
                .rearrange("(j p) c -> p j c", p=P).bitcast(MT),
            )
            psk = psK_pool.tile([P, S], F32, name="psk")    # 4 banks
            for e in range(NE):
                for sc in range(4):
                    nc.tensor.matmul(
                        psk[:, sc * 512:(sc + 1) * 512],
                        wk[:, e, :],
                        xT[:, e, sc * 512:(sc + 1) * 512],
                        start=(e == 0), stop=(e == NE - 1),
                    )
            if evict_mix and dt % 2 == 1:
                nc.vector.tensor_copy(kT[:, dt, :], psk[:])
            else:
                nc.scalar.copy(kT[:, dt, :], psk[:])
        psK_pool.release()
        wk_pool.release()
        xT_pool.release()

        # ---------------- Attention, one 128-row q-tile at a time ----------
        qT_view = qT_dram[:].rearrange("(j p) s -> p j s", p=P)
        qt_pool = tc.alloc_tile_pool(name="qtp", bufs=3 if deep_bufs else 2)
        pT_pool = tc.alloc_tile_pool(name="pTp", bufs=2)
        psS_pool = tc.alloc_tile_pool(name="psS", bufs=2, space="PSUM")
        psT2_pool = tc.alloc_tile_pool(name="psT2", bufs=2, space="PSUM")
        psO_pool = tc.alloc_tile_pool(name="psO", bufs=1, space="PSUM")
        u_pool = tc.alloc_tile_pool(name="up", bufs=2)
        mi_pool = tc.alloc_tile_pool(name="mip", bufs=4 if deep_bufs else 2)
        p_pool = tc.alloc_tile_pool(name="pp", bufs=2)
        den_pool = tc.alloc_tile_pool(name="denp", bufs=2)
        out_pool = tc.alloc_tile_pool(name="outp", bufs=2)

        for qi in (range(NS) if ablate != "no_attn" else ()):
            qt = qt_pool.tile([P, ND, P], MT, name="qt")
            getattr(nc, qt_eng).dma_start(qt[:], qT_view[:, :, qi * P:(qi + 1) * P])
            pT = pT_pool.tile([P, NS, P], MT, name="pT")    # 8KB/part
            den = den_pool.tile([P, 1], F32, name="den")
            u_full = u_pool.tile([P, S], F32, name="u_full")
            p_full = p_pool.tile([P, S], F32, name="p_full")
            if big_dma and ablate != "no_mask":
                mi_full = mi_pool.tile([P, S], MDT, name="mi_full")
                getattr(nc, mask_eng).dma_start(
                    mi_full[:], mask_d[qi * P:(qi + 1) * P, :]
                )
            for kh in range(KH):
                pss = psS_pool.tile([P, 1024], F32, name="pss")   # 2 banks
                for e in range(ND):
                    for kc in range(2):
                        nc.tensor.matmul(
                            pss[:, kc * 512:(kc + 1) * 512],
                            qt[:, e, :],
                            kT[:, e, kh * 1024 + kc * 512:
                                     kh * 1024 + (kc + 1) * 512],
                            start=(e == 0), stop=(e == ND - 1),
                        )
                # u = s/sqrt(D) + BIAS  (PSUM eviction + affine in one ACT op)
                u = u_full[:, kh * 1024:(kh + 1) * 1024]
                nc.scalar.activation(
                    u, pss[:], mybir.ActivationFunctionType.Copy,
                    bias=float(BIAS), scale=float(SCALE),
                )
                if ablate != "no_mask":
                    if big_dma:
                        nc.vector.tensor_mul(
                            u, u, mi_full[:, kh * 1024:(kh + 1) * 1024]
                        )
                    else:
                        mi = mi_pool.tile([P, 1024], MDT, name="mi")
                        getattr(nc, mask_eng).dma_start(
                            mi[:],
                            mask_d[qi * P:(qi + 1) * P,
                                   kh * 1024:(kh + 1) * 1024],
                        )
                        nc.vector.tensor_mul(u, u, mi[:])
            # p = exp(u - BIAS) over the full row, row-sum accumulated
            nc.scalar.activation(
                p_full[:], u_full[:], mybir.ActivationFunctionType.Exp,
                bias=nbias[:], scale=1.0,
                accum_out=den[:],
            )
            # pT chunks via PE transpose
            for g in range(4):
                pst = psT2_pool.tile([P, 4, P], F32, name="pst")  # 1 bank
                for jj in range(4):
                    j = g * 4 + jj
                    nc.tensor.transpose(
                        pst[:, jj, :], p_full[:, j * P:(j + 1) * P], ident[:]
                    )
                nc.vector.tensor_copy(
                    pT[:, g * 4:(g + 1) * 4, :], pst[:],
                )
            rden = den_pool.tile([P, 1], F32, name="rden")
            nc.vector.reciprocal(rden[:], den[:])
            pso = psO_pool.tile([P, D], F32, name="pso")    # 2 banks
            for dh in range(2):
                for t in range(NS):
                    nc.tensor.matmul(
                        pso[:, dh * 512:(dh + 1) * 512],
                        pT[:, t, :],
                        v_big[:, t, dh * 512:(dh + 1) * 512],
                        start=(t == 0), stop=(t == NS - 1),
                    )
            out_sb = out_pool.tile([P, D], F32, name="out_sb")
            nc.scalar.activation(
                out_sb[:], pso[:], mybir.ActivationFunctionType.Copy,
                bias=0.0, scale=rden[:],
            )
            nc.sync.dma_start(out_d[qi * P:(qi + 1) * P, :], out_sb[:])

        if ablate == "no_attn":
            dump = out_pool.tile([P, D], F32, name="dump")
            nc.vector.tensor_copy(dump[:, 0:512], kT[:, 0, 0:512].bitcast(F32))
            nc.vector.tensor_copy(dump[:, 512:1024],
                                  v_big[:, 0, 0:512].bitcast(F32))
            nc.sync.dma_start(out_d[0:P, :], dump[:])

        for pool in (out_pool, den_pool, p_pool, mi_pool,
                     u_pool, psO_pool, psT2_pool, psS_pool,
                     pT_pool, qt_pool, kT_pool, v_pool, const_pool):
            pool.release()

    nc.finalize()
    return nc


_program_cache = {}


def _get_program():
    key = MM_DTYPE
    if key not in _program_cache:
        _program_cache[key] = build_program()
    return _program_cache[key]


def kernel(x, mask, Wq, Wk, Wv):
    x = np.ascontiguousarray(np.asarray(x, dtype=np.float32))
    mask = np.ascontiguousarray(np.asarray(mask).astype(np.int8))
    Wq = np.ascontiguousarray(np.asarray(Wq, dtype=np.float32))
    Wk = np.ascontiguousarray(np.asarray(Wk, dtype=np.float32))
    Wv = np.ascontiguousarray(np.asarray(Wv, dtype=np.float32))
    ident = np.eye(P, dtype=np.float32)

    nc = _get_program()
    in_maps = [
        {"x": x[b], "mask": mask[b], "Wq": Wq, "Wk": Wk, "Wv": Wv,
         "ident": ident}
        for b in range(B)
    ]
    res = run_bass_kernel_spmd(nc, in_maps, list(range(B))).results
    return np.stack([res[b]["out"] for b in range(B)], axis=0)
